# revision 47
# baseline (speedup 1.0000x reference)
"""AttnBlock (GroupNorm -> QKV 1x1 -> single-head attention over 4096 tokens
-> proj -> residual) on 8 Trainium2 NeuronCores, data-parallel over batch.

Per-core layout strategy (one image per core, N=4096 tokens, C=512 channels):
  - x loaded token-major (f32), converted to bf16; GroupNorm stats via
    ones-lhsT colsum matmuls in token-major; GroupNorm folded into the QKV
    weights (row scale) and effective biases, so hT is just x transposed.
  - x PE-transposed (bf16) to channel-major hT.
  - qT, kT computed channel-major (lhsT = w tiles, rhs = hT), v token-major
    (lhsT = hT tiles, rhs = wv).  All three stay RESIDENT in SBUF for the
    whole attention phase (bf16 or fp8e4) - no DRAM spill/restream.
  - Attention per query group of 512 queries: scores^T tiles [j=128, i=512]
    (lhsT = kT, rhs = qT slice), exp on ACT directly into SBUF; denominator
    via ones-lhsT matmuls accumulated in PSUM alongside the AV matmuls;
    flash-style AV accumulation in PSUM over j tiles.
    In fp8 mode the scores and AV matmuls run in fp8e4 with
    perf_mode=DoubleRow (contraction 256 per matmul, 0.5 cycles/row), and
    exp is computed as exp(s/sqrt(C) - 3 ln 2) to keep values in e4m3 range
    (the 2^-3 factor cancels in the softmax ratio).
  - proj: lhsT = AV^T tiles (copied PSUM->SBUF bf16), rhs = wo (bf16);
    softmax normalization (1/denominator, transposed to per-partition via a
    DRAM bounce) and residual+bias applied in one DVE scalar_tensor_tensor.
"""

import numpy as np

import concourse.bass as bass
import concourse.tile as tile
from concourse import bacc, mybir
from concourse.bass_utils import run_bass_kernel_spmd

B, H, W, C = 8, 64, 64, 512
N = H * W            # 4096 tokens per image
G = 32               # groups
EPS = 1e-5
N_CORES = 8

F32 = mybir.dt.float32
F32R = mybir.dt.float32r
BF16 = mybir.dt.bfloat16
FP8 = mybir.dt.float8e4
AF = mybir.ActivationFunctionType
ALU = mybir.AluOpType
DR = mybir.MatmulPerfMode.DoubleRow

NT = N // 128        # 32 token tiles
CT = C // 128        # 4 channel tiles
NG = 8               # query groups
GW = N // NG         # 512 queries per group
NB = GW // 128       # 4 token blocks per group
CPG = C // G         # 16 channels per group

EXP_BIAS = float(-3.0 * np.log(2.0))  # keep exp outputs < 240 for e4m3


def build_program(reps: int = 1, attn: str = "fp8", phases: str = "ABC"):
    att_dt = FP8 if attn.startswith("fp8") else BF16
    qkv8 = attn in ("fp8x", "fp8xx")   # fp8 DoubleRow QKV projections
    h_dt = FP8 if qkv8 else BF16
    nc = bacc.Bacc("TRN2", target_bir_lowering=False, debug=False,
                   num_devices=N_CORES)

    x_ap = nc.dram_tensor("x", [N, C], F32, kind="ExternalInput").ap()
    wq_ap = nc.dram_tensor("wq", [C, C], F32, kind="ExternalInput").ap()
    wk_ap = nc.dram_tensor("wk", [C, C], F32, kind="ExternalInput").ap()
    wv_ap = nc.dram_tensor("wv", [C, C], F32, kind="ExternalInput").ap()
    wo_ap = nc.dram_tensor("wo", [C, C], F32, kind="ExternalInput").ap()
    bq_ap = nc.dram_tensor("bq", [C], F32, kind="ExternalInput").ap()
    bk_ap = nc.dram_tensor("bk", [C], F32, kind="ExternalInput").ap()
    bv_ap = nc.dram_tensor("bv", [C], F32, kind="ExternalInput").ap()
    bo_ap = nc.dram_tensor("bo", [C], F32, kind="ExternalInput").ap()
    gns_ap = nc.dram_tensor("gn_scale", [C], F32, kind="ExternalInput").ap()
    gnb_ap = nc.dram_tensor("gn_bias", [C], F32, kind="ExternalInput").ap()
    id_ap = nc.dram_tensor("ident", [128, 128], F32, kind="ExternalInput").ap()
    out_ap = nc.dram_tensor("out", [N, C], F32, kind="ExternalOutput").ap()

    gn_bounce = nc.dram_tensor("gn_bounce", [2, C], F32).ap()
    bias_bounce = nc.dram_tensor("bias_bounce", [3, C], F32).ap()
    den_bounce = nc.dram_tensor("den_bounce", [NG, GW], F32).ap()

    x_r = x_ap.rearrange("(nt p) c -> nt p c", p=128)
    out_r = out_ap.rearrange("(nt p) c -> nt p c", p=128)

    with tile.TileContext(nc) as tc, \
         nc.allow_low_precision(reason="bf16/fp8 attention by design"):
        rep_ctx = tc.For_i(0, reps, 1) if reps > 1 else None
        import contextlib
        with contextlib.ExitStack() as st:
            if rep_ctx is not None:
                st.enter_context(rep_ctx)
            consts = st.enter_context(tc.tile_pool(name="consts", bufs=1))
            big = st.enter_context(tc.tile_pool(name="big", bufs=1))
            res = st.enter_context(tc.tile_pool(name="res", bufs=1))
            small = st.enter_context(tc.tile_pool(name="small", bufs=1))

            # ---- constants -------------------------------------------------
            id_raw = consts.tile([128, 128], F32, tag="id_raw")
            nc.sync.dma_start(id_raw[:], id_ap[:])
            id_bf = consts.tile([128, 128], BF16, tag="id_bf")
            nc.vector.tensor_copy(id_bf[:], id_raw[:])
            ones_f = consts.tile([128, 1], F32, tag="ones_f")
            nc.vector.memset(ones_f[:], 1.0)
            ones_bf = consts.tile([128, 1], BF16, tag="ones_bf")
            nc.vector.tensor_copy(ones_bf[:], ones_f[:])
            eb_t = consts.tile([128, 1], F32, tag="eb_t")
            nc.vector.memset(eb_t[:], EXP_BIAS)
            # [128, 2, 16] so the k-tile step is 16B (dual-fp8 LDW rule);
            # only [:, :, 0:1] is used as the DoubleRow ones lhsT
            ones8_t = consts.tile([128, 2, 16], FP8, tag="ones8")
            nc.vector.memset(ones8_t[:], 1.0)
            ones8 = ones8_t[:, :, 0:1]

            w_bf = {}
            w_dma = {"wq": nc.gpsimd, "wk": nc.sync, "wv": nc.scalar,
                     "wo": nc.gpsimd}
            for name, ap in (("wq", wq_ap), ("wk", wk_ap), ("wv", wv_ap),
                             ("wo", wo_ap)):
                raw = small.tile([128, CT, C], F32, tag="w_raw")
                w_dma[name].dma_start(raw[:],
                                      ap.rearrange("(ct p) d -> p ct d", p=128))
                wr = consts.tile([128, CT, C], BF16, tag=f"{name}_bf")
                nc.vector.tensor_copy(wr[:], raw[:])
                w_bf[name] = wr

            bqk_direct = consts.tile([128, 2, CT], F32, tag="bqk_direct")
            nc.sync.dma_start(bqk_direct[:, 0, :],
                              bq_ap.rearrange("(dt p) -> p dt", p=128))
            nc.sync.dma_start(bqk_direct[:, 1, :],
                              bk_ap.rearrange("(dt p) -> p dt", p=128))
            bv_direct = consts.tile([128, C], F32, tag="bv_direct")
            nc.sync.dma_start(bv_direct[:],
                              bv_ap.unsqueeze(0).partition_broadcast(128))
            bqkT = consts.tile([128, 2, CT], F32, tag="bqkT")
            bvb = consts.tile([128, C], F32, tag="bvb")
            bob = consts.tile([128, C], F32, tag="bob")
            nc.sync.dma_start(bob[:], bo_ap.unsqueeze(0).partition_broadcast(128))
            gns_sb = consts.tile([1, C], F32, tag="gns")
            nc.sync.dma_start(gns_sb[:], gns_ap.unsqueeze(0))
            gnb_sb = consts.tile([1, C], F32, tag="gnb")
            nc.sync.dma_start(gnb_sb[:], gnb_ap.unsqueeze(0))

            # resident attention operands
            kT = big.tile([128, CT, N], att_dt, tag="kT")
            qT = res.tile([128, CT, N], att_dt, tag="qT")
            vtm = res.tile([128, NT, C], att_dt, tag="v")
            # one group's exp(scores) tiles, resident (enables 2-pass AV)
            a_all = None
            if attn.startswith("fp8"):
                a_all = res.tile([128, NT // 2, 2, GW], FP8, tag="a_all",
                                 name="a_all")

            w8 = {}
            if qkv8:
                for name in ("wq", "wk", "wv"):
                    w8[name] = consts.tile([128, CT, C], FP8, tag=f"{name}8",
                                           name=f"{name}8")

            with tc.tile_pool(name="hTp", bufs=1) as hTp:
                hT = hTp.tile([128, CT, N], h_dt, tag="hT")

                # ---- phase A: load x, stats, transpose -------------------
                xr_all = big.tile([128, NT, C], BF16, tag="xr")
                with (
                    tc.tile_pool(name="pa_ps", bufs=1, space=bass.MemorySpace.PSUM) as paps,
                    tc.tile_pool(name="pa_tps", bufs=4, space=bass.MemorySpace.PSUM) as patps,
                    tc.tile_pool(name="xin", bufs=3) as xin,
                    tc.tile_pool(name="x2p", bufs=1) as x2p,
                ):
                    s1_ps = paps.tile([1, C], F32, tag="s1")
                    s2_ps = paps.tile([1, C], F32, tag="s2")
                    CH = 4  # nt per chunk
                    x_dma = (nc.sync, nc.scalar, nc.gpsimd)
                    for ch in range(NT // CH):
                        x_t = xin.tile([128, CH, C], F32, tag="x_t")
                        x_dma[ch % 3].dma_start(
                            x_t[:], x_r[ch * CH:(ch + 1) * CH].transpose([1, 0, 2]))
                        xr_t = xr_all[:, ch * CH:(ch + 1) * CH, :]
                        nc.vector.tensor_copy(xr_t, x_t[:])
                        x2_t = x2p.tile([128, CH, C], BF16, tag="x2_t")
                        nc.scalar.activation(x2_t[:], x_t[:], AF.Square)
                        for u in range(CH):
                            nt = ch * CH + u
                            nc.tensor.matmul(s1_ps[:], ones_bf[:], xr_all[:, nt, :],
                                             start=(nt == 0), stop=(nt == NT - 1))
                            nc.tensor.matmul(s2_ps[:], ones_bf[:], x2_t[:, u, :],
                                             start=(nt == 0), stop=(nt == NT - 1))
                        # transpose this chunk (4 nt x 4 ct) while the next
                        # chunk's DMA is in flight
                        for ct in range(CT):
                            tp = patps.tile([128, 512], BF16, tag="tp")
                            for u in range(CH):
                                nt = ch * CH + u
                                nc.tensor.transpose(
                                    tp[:, bass.ts(u, 128)],
                                    xr_all[:, nt, bass.ts(ct, 128)], id_bf[:])
                            nc.vector.tensor_copy(
                                hT[:, ct, ch * 512:(ch + 1) * 512], tp[:])

                    # group stats on partition 0
                    g1 = small.tile([1, G], F32, tag="g1")
                    nc.vector.reduce_sum(
                        g1[:], s1_ps[:].rearrange("p (g k) -> p g k", k=CPG),
                        axis=mybir.AxisListType.X)
                    g2 = small.tile([1, G], F32, tag="g2")
                    nc.vector.reduce_sum(
                        g2[:], s2_ps[:].rearrange("p (g k) -> p g k", k=CPG),
                        axis=mybir.AxisListType.X)
                    cnt = 1.0 / (N * CPG)
                    mean = small.tile([1, G], F32, tag="mean")
                    nc.scalar.mul(mean[:], g1[:], cnt)
                    ex2 = small.tile([1, G], F32, tag="ex2")
                    nc.scalar.mul(ex2[:], g2[:], cnt)
                    var = small.tile([1, G], F32, tag="var")
                    nc.vector.tensor_tensor(var[:], mean[:], mean[:], op=ALU.mult)
                    nc.vector.tensor_tensor(var[:], ex2[:], var[:], op=ALU.subtract)
                    eps_t = small.tile([1, 1], F32, tag="eps_t")
                    nc.vector.memset(eps_t[:], EPS)
                    sd = small.tile([1, G], F32, tag="sd")
                    nc.scalar.activation(sd[:], var[:], AF.Sqrt, bias=eps_t[:])
                    inv = small.tile([1, G], F32, tag="inv")
                    nc.vector.reciprocal(inv[:], sd[:])
                    # broadcast group -> channel (free-dim stride-0 read)
                    invc = small.tile([1, C], F32, tag="invc")
                    nc.vector.tensor_copy(
                        invc[:].rearrange("p (g k) -> p g k", k=CPG),
                        inv[:].unsqueeze(2).broadcast_to([1, G, CPG]))
                    meanc = small.tile([1, C], F32, tag="meanc")
                    nc.vector.tensor_copy(
                        meanc[:].rearrange("p (g k) -> p g k", k=CPG),
                        mean[:].unsqueeze(2).broadcast_to([1, G, CPG]))
                    a_c = small.tile([1, C], F32, tag="a_c")
                    nc.vector.tensor_tensor(a_c[:], invc[:], gns_sb[:], op=ALU.mult)
                    b_c = small.tile([1, C], F32, tag="b_c")
                    nc.vector.tensor_tensor(b_c[:], meanc[:], a_c[:], op=ALU.mult)
                    nc.vector.tensor_tensor(b_c[:], gnb_sb[:], b_c[:], op=ALU.subtract)
                    # bounce [1, C] -> per-partition [128, 2, CT]
                    nc.sync.dma_start(gn_bounce[0].unsqueeze(0), a_c[:])
                    nc.sync.dma_start(gn_bounce[1].unsqueeze(0), b_c[:])
                    ab_sb = small.tile([128, 2, CT], F32, tag="ab_sb")
                    nc.sync.dma_start(
                        ab_sb[:], gn_bounce.rearrange("two (ct p) -> p two ct", p=128))
                    b_rT = small.tile([128, CT], BF16, tag="b_rT")
                    nc.vector.tensor_copy(b_rT[:], ab_sb[:, 1, :])
                    # scaled weights in-place: w = a_c (row scale) * w.
                    # Tile orders these after the bias matmuls below, which
                    # read the raw weights (WAR on the same tile).
                    def _scale_weights():
                        for name in ("wq", "wk", "wv"):
                            for ct in range(CT):
                                dst = (w8[name][:, ct, :] if qkv8
                                       else w_bf[name][:, ct, :])
                                nc.scalar.activation(
                                    dst, w_bf[name][:, ct, :],
                                    AF.Copy, bias=0.0,
                                    scale=ab_sb[:, 0, ct:ct + 1])
                    # effective biases: b_c @ w + orig_bias
                    with tc.tile_pool(name="bps", bufs=1,
                                      space=bass.MemorySpace.PSUM) as bps:
                        for i, name in enumerate(("wq", "wk", "wv")):
                            bp = bps.tile([1, C], F32, tag="bp", name=f"bp{i}")
                            for ct in range(CT):
                                nc.tensor.matmul(
                                    bp[:], b_rT[:, ct:ct + 1], w_bf[name][:, ct, :],
                                    start=(ct == 0), stop=(ct == CT - 1))
                            btmp = small.tile([1, C], F32, tag="btmp",
                                              name=f"btmp{i}")
                            nc.vector.tensor_copy(btmp[:], bp[:])
                            nc.sync.dma_start(bias_bounce[i].unsqueeze(0), btmp[:])
                            if name == "wv":
                                nc.sync.dma_start(
                                    bvb[:],
                                    bias_bounce[i].unsqueeze(0).partition_broadcast(128))
                                nc.vector.tensor_tensor(bvb[:], bvb[:],
                                                        bv_direct[:], op=ALU.add)
                            else:
                                nc.sync.dma_start(
                                    bqkT[:, i, :],
                                    bias_bounce[i].rearrange("(dt p) -> p dt", p=128))
                                nc.vector.tensor_tensor(
                                    bqkT[:, i, :], bqkT[:, i, :],
                                    bqk_direct[:, i, :], op=ALU.add)
                        _scale_weights()

                # ---- phase B: q^T, k^T, v (all SBUF-resident) -------------
                if "B" in phases:
                  with (
                    tc.tile_pool(name="pb_ps", bufs=4, space=bass.MemorySpace.PSUM) as pbps,
                  ):
                    def _qk_mms(ps, wname, nb2, dt, half):
                        lo = nb2 * 1024 + half * 512
                        if qkv8:
                            for cp in range(CT // 2):
                                nc.tensor.matmul(
                                    ps[:, bass.ts(half, 512)],
                                    w8[wname][:, 2 * cp:2 * cp + 2, bass.ts(dt, 128)],
                                    hT[:, 2 * cp:2 * cp + 2, lo:lo + 512],
                                    start=(cp == 0), stop=(cp == CT // 2 - 1),
                                    perf_mode=DR)
                        else:
                            for ct in range(CT):
                                nc.tensor.matmul(
                                    ps[:, bass.ts(half, 512)],
                                    w_bf[wname][:, ct, bass.ts(dt, 128)],
                                    hT[:, ct, lo:lo + 512],
                                    start=(ct == 0), stop=(ct == CT - 1))

                    for nb2 in range(N // 1024):
                        for dt in range(CT):
                            k_ps = pbps.tile([128, 1024], F32, tag="qkv_ps")
                            for half in range(2):
                                _qk_mms(k_ps, "wk", nb2, dt, half)
                            nc.vector.tensor_scalar(
                                kT[:, dt, bass.ts(nb2, 1024)], k_ps[:],
                                1.0, bqkT[:, 1, dt:dt + 1],
                                op0=ALU.mult, op1=ALU.add)

                            q_ps = pbps.tile([128, 1024], F32, tag="qkv_ps")
                            for half in range(2):
                                _qk_mms(q_ps, "wq", nb2, dt, half)
                            nc.scalar.activation(qT[:, dt, bass.ts(nb2, 1024)],
                                                 q_ps[:], AF.Identity,
                                                 bias=bqkT[:, 0, dt:dt + 1])
                    for nt2 in range(NT // 2):
                        v_ps = pbps.tile([128, 1024], F32, tag="qkv_ps")
                        for half in range(2):
                            nt = nt2 * 2 + half
                            if qkv8:
                                for cp in range(CT // 2):
                                    nc.tensor.matmul(
                                        v_ps[:, bass.ts(half, 512)],
                                        hT[:, 2 * cp:2 * cp + 2, bass.ts(nt, 128)],
                                        w8["wv"][:, 2 * cp:2 * cp + 2, :],
                                        start=(cp == 0), stop=(cp == CT // 2 - 1),
                                        perf_mode=DR)
                            else:
                                for ct in range(CT):
                                    nc.tensor.matmul(
                                        v_ps[:, bass.ts(half, 512)],
                                        hT[:, ct, bass.ts(nt, 128)],
                                        w_bf["wv"][:, ct, :],
                                        start=(ct == 0), stop=(ct == CT - 1))
                        nc.vector.tensor_tensor(
                            vtm[:, nt2 * 2:nt2 * 2 + 2, :],
                            v_ps[:].rearrange("p (two c) -> p two c", two=2),
                            bvb[:].unsqueeze(1).broadcast_to([128, 2, C]),
                            op=ALU.add)

            # ---- phase C: attention + proj + residual ----------------------
            if "C" not in phases:
                with tc.tile_pool(name="dummy_out", bufs=1) as dop:
                    d_t = dop.tile([128, C], F32, tag="d_t")
                    nc.vector.memset(d_t[:], 0.0)
                    for nt in range(NT):
                        nc.sync.dma_start(out_r[nt], d_t[:])
            from collections import deque
            if "C" in phases:
              n_s_bufs = 5 if attn.startswith("fp8") else 3
              n_av_bufs = 2 if attn.startswith("fp8") else CT
              with (
                tc.tile_pool(name="pc_s", bufs=n_s_bufs, space=bass.MemorySpace.PSUM) as pcs,
                tc.tile_pool(name="pc_av", bufs=n_av_bufs, space=bass.MemorySpace.PSUM) as pcav,
                tc.tile_pool(name="pc_o", bufs=1, space=bass.MemorySpace.PSUM) as pco,
                tc.tile_pool(name="atp", bufs=4) as atp,
                tc.tile_pool(name="avtp", bufs=2) as avtp,
                tc.tile_pool(name="xbp", bufs=4) as xbp,
                tc.tile_pool(name="obp", bufs=3) as obp,
                tc.tile_pool(name="rp", bufs=2) as rp,
            ):
                def _make_finalize(g, avT, r_sb):
                    # proj + residual for group g, issued early in group g+1
                    # (the av banks are free between avT copy-out and the next
                    # group's first AV matmul, so o_ps borrows them instead of
                    # contending with the score tiles)
                    def _fin():
                        for nb in range(NB):
                            o_ps = pcav.tile([128, C], F32, tag="av",
                                             name=f"o_ps_{g}_{nb}")
                            for dt in range(CT):
                                nc.tensor.matmul(
                                    o_ps[:],
                                    avT[:, dt, bass.ts(nb, 128)],
                                    w_bf["wo"][:, dt, :],
                                    start=(dt == 0), stop=(dt == CT - 1))
                            nt = g * NB + nb
                            o_sb = obp.tile([128, C], F32, tag="o_sb")
                            nc.vector.tensor_copy(o_sb[:], o_ps[:])
                            xb = xbp.tile([128, C], F32, tag="xb")
                            nc.sync.dma_start(xb[:], x_r[nt])
                            nc.vector.tensor_tensor(xb[:], xb[:], bob[:],
                                                    op=ALU.add)
                            ob = obp.tile([128, C], F32, tag="ob")
                            nc.vector.scalar_tensor_tensor(
                                ob[:], o_sb[:], r_sb[:, nb:nb + 1], xb[:],
                                op0=ALU.mult, op1=ALU.add)
                            nc.sync.dma_start(out_r[nt], ob[:])
                    return _fin

                prev_fin = None
                for g in range(NG):
                    av_ps = []

                    def _ensure_av(g=g, av_ps=av_ps):
                        # allocated lazily, after the previous group's o_ps
                        # tiles borrowed these banks in _fin
                        if not av_ps:
                            av_ps.extend(
                                pcav.tile([128, GW], F32, tag="av",
                                          name=f"av_{g}_{dt}")
                                for dt in range(CT))

                    den_ps = pco.tile([1, GW], F32, tag="den_ps", name="den_ps")
                    pend_q = deque()

                    if attn.startswith("fp8"):
                        # pass 1: AV for dt 0,1 (2 PSUM banks) + denominator,
                        # issued alongside the scores/exp stream
                        def _issue_av(pa, pj, last, _ensure_av=_ensure_av):
                            _ensure_av()
                            nc.tensor.matmul(den_ps[:], ones8, pa[:],
                                             start=(pj == 0), stop=last,
                                             perf_mode=DR)
                            for dt in range(2):
                                nc.tensor.matmul(
                                    av_ps[dt][:],
                                    vtm[:, 2 * pj:2 * pj + 2, bass.ts(dt, 128)],
                                    pa[:],
                                    start=(pj == 0), stop=last,
                                    perf_mode=DR)

                        for jtp in range(NT // 2):
                            a_pair = a_all[:, jtp]
                            for u2 in range(2):
                                jt = 2 * jtp + u2
                                s_ps = pcs.tile([128, GW], F32, tag="s_ps")
                                for cp in range(CT // 2):
                                    nc.tensor.matmul(
                                        s_ps[:],
                                        kT[:, 2 * cp:2 * cp + 2, bass.ts(jt, 128)],
                                        qT[:, 2 * cp:2 * cp + 2, bass.ts(g, GW)],
                                        start=(cp == 0), stop=(cp == 1),
                                        perf_mode=DR)
                                nc.scalar.activation(a_pair[:, u2, :], s_ps[:],
                                                     AF.Exp,
                                                     scale=float(C) ** -0.5,
                                                     bias=eb_t[:])
                            pend_q.append((a_pair, jtp))
                            if jtp == 2 and prev_fin is not None:
                                prev_fin()
                                prev_fin = None
                            if len(pend_q) > 2:
                                pa, pj = pend_q.popleft()
                                _issue_av(pa, pj, False)
                        while pend_q:
                            pa, pj = pend_q.popleft()
                            _issue_av(pa, pj, not pend_q)
                        # copy out the finished dt 0,1 AV halves, freeing
                        # their banks for pass 2
                        avT = avtp.tile([128, CT, GW], BF16, tag="avT")
                        for dt in range(2):
                            nc.vector.tensor_copy(avT[:, dt, :], av_ps[dt][:])
                        # pass 2: AV for dt 2,3 over the resident a_all
                        av_ps2 = [pcav.tile([128, GW], F32, tag="av",
                                            name=f"av2_{g}_{dt}")
                                  for dt in range(2)]
                        for pj in range(NT // 2):
                            for i, dt in enumerate((2, 3)):
                                nc.tensor.matmul(
                                    av_ps2[i][:],
                                    vtm[:, 2 * pj:2 * pj + 2, bass.ts(dt, 128)],
                                    a_all[:, pj],
                                    start=(pj == 0), stop=(pj == NT // 2 - 1),
                                    perf_mode=DR)
                        for i, dt in enumerate((2, 3)):
                            nc.vector.tensor_copy(avT[:, dt, :], av_ps2[i][:])
                    else:
                        def _issue_av16(pa, pj, last, _ensure_av=_ensure_av):
                            _ensure_av()
                            nc.tensor.matmul(den_ps[:], ones_bf[:], pa[:],
                                             start=(pj == 0), stop=last)
                            for dt in range(CT):
                                nc.tensor.matmul(
                                    av_ps[dt][:],
                                    vtm[:, pj, bass.ts(dt, 128)],
                                    pa[:],
                                    start=(pj == 0), stop=last)

                        for jt in range(NT):
                            s_ps = pcs.tile([128, GW], F32, tag="s_ps")
                            for ct in range(CT):
                                nc.tensor.matmul(
                                    s_ps[:],
                                    kT[:, ct, bass.ts(jt, 128)],
                                    qT[:, ct, bass.ts(g, GW)],
                                    start=(ct == 0), stop=(ct == CT - 1))
                            a_t = atp.tile([128, GW], BF16, tag="a_pair")
                            nc.scalar.activation(a_t[:], s_ps[:], AF.Exp,
                                                 scale=float(C) ** -0.5)
                            pend_q.append((a_t, jt))
                            if jt == 4 and prev_fin is not None:
                                prev_fin()
                                prev_fin = None
                            if len(pend_q) > 3:
                                pa, pj = pend_q.popleft()
                                _issue_av16(pa, pj, False)
                        while pend_q:
                            pa, pj = pend_q.popleft()
                            _issue_av16(pa, pj, not pend_q)

                        # AV^T -> SBUF (bf16) for proj lhsT
                        avT = avtp.tile([128, CT, GW], BF16, tag="avT")
                        for dt in range(CT):
                            nc.vector.tensor_copy(avT[:, dt, :], av_ps[dt][:])

                    # reciprocal -> per-partition via DRAM bounce
                    recip = rp.tile([1, GW], F32, tag="recip")
                    nc.vector.reciprocal(recip[:], den_ps[:])
                    nc.sync.dma_start(den_bounce[g].unsqueeze(0), recip[:])
                    r_sb = rp.tile([128, NB], F32, tag="r_sb")
                    nc.sync.dma_start(
                        r_sb[:], den_bounce[g].rearrange("(nb p) -> p nb", p=128))
                    prev_fin = _make_finalize(g, avT, r_sb)
                prev_fin()

    nc.compile()
    return nc


_CACHE = {}


def _get_program(reps: int = 1, attn: str = "fp8", phases: str = "ABC"):
    key = (reps, attn, phases)
    if key not in _CACHE:
        _CACHE[key] = build_program(reps, attn, phases)
    return _CACHE[key]


def make_in_maps(inputs):
    ident = np.eye(128, dtype=np.float32)
    x = np.asarray(inputs["x"], dtype=np.float32).reshape(B, N, C)
    shared = {k: np.ascontiguousarray(np.asarray(inputs[k], dtype=np.float32))
              for k in ("wq", "wk", "wv", "wo", "bq", "bk", "bv", "bo",
                        "gn_scale", "gn_bias")}
    return [dict(x=np.ascontiguousarray(x[c]), ident=ident, **shared)
            for c in range(N_CORES)]


DEFAULT_ATTN = "fp8x"


def kernel(**inputs) -> np.ndarray:
    nc = _get_program(1, DEFAULT_ATTN)
    in_maps = make_in_maps(inputs)
    last_err = None
    for _attempt in range(3):
        try:
            res = run_bass_kernel_spmd(nc, in_maps, list(range(N_CORES)))
            break
        except Exception as e:  # transient NRT device errors recover on retry
            last_err = e
    else:
        raise last_err
    out = np.stack([res.results[c]["out"] for c in range(N_CORES)], axis=0)
    return out.reshape(B, H, W, C)


# revision 48
# speedup vs baseline: 1.0945x; 1.0945x over previous
"""AttnBlock (GroupNorm -> QKV 1x1 -> single-head attention over 4096 tokens
-> proj -> residual) on 8 Trainium2 NeuronCores, data-parallel over batch.

Per-core layout strategy (one image per core, N=4096 tokens, C=512 channels):
  - x loaded token-major (f32), converted to bf16; GroupNorm stats via
    ones-lhsT colsum matmuls in token-major; GroupNorm folded into the QKV
    weights (row scale) and effective biases, so hT is just x transposed.
  - x PE-transposed (bf16) to channel-major hT.
  - qT, kT computed channel-major (lhsT = w tiles, rhs = hT), v token-major
    (lhsT = hT tiles, rhs = wv).  All three stay RESIDENT in SBUF for the
    whole attention phase (bf16 or fp8e4) - no DRAM spill/restream.
  - Attention per query group of 512 queries: scores^T tiles [j=128, i=512]
    (lhsT = kT, rhs = qT slice), exp on ACT directly into SBUF; denominator
    via ones-lhsT matmuls accumulated in PSUM alongside the AV matmuls;
    flash-style AV accumulation in PSUM over j tiles.
    In fp8 mode the scores and AV matmuls run in fp8e4 with
    perf_mode=DoubleRow (contraction 256 per matmul, 0.5 cycles/row), and
    exp is computed as exp(s/sqrt(C) - 3 ln 2) to keep values in e4m3 range
    (the 2^-3 factor cancels in the softmax ratio).
  - proj: lhsT = AV^T tiles (copied PSUM->SBUF bf16), rhs = wo (bf16);
    softmax normalization (1/denominator, transposed to per-partition via a
    DRAM bounce) and residual+bias applied in one DVE scalar_tensor_tensor.
"""

import numpy as np

import concourse.bass as bass
import concourse.tile as tile
from concourse import bacc, mybir
from concourse.bass_utils import run_bass_kernel_spmd

B, H, W, C = 8, 64, 64, 512
N = H * W            # 4096 tokens per image
G = 32               # groups
EPS = 1e-5
N_CORES = 8

F32 = mybir.dt.float32
F32R = mybir.dt.float32r
BF16 = mybir.dt.bfloat16
FP8 = mybir.dt.float8e4
AF = mybir.ActivationFunctionType
ALU = mybir.AluOpType
DR = mybir.MatmulPerfMode.DoubleRow

NT = N // 128        # 32 token tiles
CT = C // 128        # 4 channel tiles
NG = 8               # query groups
GW = N // NG         # 512 queries per group
NB = GW // 128       # 4 token blocks per group
CPG = C // G         # 16 channels per group

EXP_BIAS = float(-3.0 * np.log(2.0))  # keep exp outputs < 240 for e4m3


def build_program(reps: int = 1, attn: str = "fp8", phases: str = "ABC"):
    att_dt = FP8 if attn.startswith("fp8") else BF16
    qkv8 = attn in ("fp8x", "fp8xx")   # fp8 DoubleRow QKV projections
    h_dt = FP8 if qkv8 else BF16
    nc = bacc.Bacc("TRN2", target_bir_lowering=False, debug=False,
                   num_devices=N_CORES)

    x_ap = nc.dram_tensor("x", [N, C], F32, kind="ExternalInput").ap()
    wq_ap = nc.dram_tensor("wq", [C, C], F32, kind="ExternalInput").ap()
    wk_ap = nc.dram_tensor("wk", [C, C], F32, kind="ExternalInput").ap()
    wv_ap = nc.dram_tensor("wv", [C, C], F32, kind="ExternalInput").ap()
    wo_ap = nc.dram_tensor("wo", [C, C], F32, kind="ExternalInput").ap()
    bq_ap = nc.dram_tensor("bq", [C], F32, kind="ExternalInput").ap()
    bk_ap = nc.dram_tensor("bk", [C], F32, kind="ExternalInput").ap()
    bv_ap = nc.dram_tensor("bv", [C], F32, kind="ExternalInput").ap()
    bo_ap = nc.dram_tensor("bo", [C], F32, kind="ExternalInput").ap()
    gns_ap = nc.dram_tensor("gn_scale", [C], F32, kind="ExternalInput").ap()
    gnb_ap = nc.dram_tensor("gn_bias", [C], F32, kind="ExternalInput").ap()
    id_ap = nc.dram_tensor("ident", [128, 128], F32, kind="ExternalInput").ap()
    out_ap = nc.dram_tensor("out", [N, C], F32, kind="ExternalOutput").ap()

    gn_bounce = nc.dram_tensor("gn_bounce", [2, C], F32).ap()
    bias_bounce = nc.dram_tensor("bias_bounce", [3, C], F32).ap()
    den_bounce = nc.dram_tensor("den_bounce", [NG, GW], F32).ap()

    x_r = x_ap.rearrange("(nt p) c -> nt p c", p=128)
    out_r = out_ap.rearrange("(nt p) c -> nt p c", p=128)

    with tile.TileContext(nc) as tc, \
         nc.allow_low_precision(reason="bf16/fp8 attention by design"):
        rep_ctx = tc.For_i(0, reps, 1) if reps > 1 else None
        import contextlib
        with contextlib.ExitStack() as st:
            if rep_ctx is not None:
                st.enter_context(rep_ctx)
            consts = st.enter_context(tc.tile_pool(name="consts", bufs=1))
            big = st.enter_context(tc.tile_pool(name="big", bufs=1))
            res = st.enter_context(tc.tile_pool(name="res", bufs=1))
            small = st.enter_context(tc.tile_pool(name="small", bufs=1))

            # ---- constants -------------------------------------------------
            id_raw = consts.tile([128, 128], F32, tag="id_raw")
            nc.sync.dma_start(id_raw[:], id_ap[:])
            id_bf = consts.tile([128, 128], BF16, tag="id_bf")
            nc.vector.tensor_copy(id_bf[:], id_raw[:])
            ones_f = consts.tile([128, 1], F32, tag="ones_f")
            nc.vector.memset(ones_f[:], 1.0)
            ones_bf = consts.tile([128, 1], BF16, tag="ones_bf")
            nc.vector.tensor_copy(ones_bf[:], ones_f[:])
            eb_t = consts.tile([128, 1], F32, tag="eb_t")
            nc.vector.memset(eb_t[:], EXP_BIAS)
            # [128, 2, 16] so the k-tile step is 16B (dual-fp8 LDW rule);
            # only [:, :, 0:1] is used as the DoubleRow ones lhsT
            ones8_t = consts.tile([128, 2, 16], FP8, tag="ones8")
            nc.vector.memset(ones8_t[:], 1.0)
            ones8 = ones8_t[:, :, 0:1]

            w_bf = {}
            for name, ap in (("wq", wq_ap), ("wk", wk_ap), ("wv", wv_ap),
                             ("wo", wo_ap)):
                raw = small.tile([128, CT, C], F32, tag="w_raw")
                nc.sync.dma_start(raw[:], ap.rearrange("(ct p) d -> p ct d", p=128))
                wr = consts.tile([128, CT, C], BF16, tag=f"{name}_bf")
                nc.vector.tensor_copy(wr[:], raw[:])
                w_bf[name] = wr

            bqk_direct = consts.tile([128, 2, CT], F32, tag="bqk_direct")
            nc.sync.dma_start(bqk_direct[:, 0, :],
                              bq_ap.rearrange("(dt p) -> p dt", p=128))
            nc.sync.dma_start(bqk_direct[:, 1, :],
                              bk_ap.rearrange("(dt p) -> p dt", p=128))
            bv_direct = consts.tile([128, C], F32, tag="bv_direct")
            nc.sync.dma_start(bv_direct[:],
                              bv_ap.unsqueeze(0).partition_broadcast(128))
            bqkT = consts.tile([128, 2, CT], F32, tag="bqkT")
            bvb = consts.tile([128, C], F32, tag="bvb")
            bob = consts.tile([128, C], F32, tag="bob")
            nc.sync.dma_start(bob[:], bo_ap.unsqueeze(0).partition_broadcast(128))
            gns_sb = consts.tile([1, C], F32, tag="gns")
            nc.sync.dma_start(gns_sb[:], gns_ap.unsqueeze(0))
            gnb_sb = consts.tile([1, C], F32, tag="gnb")
            nc.sync.dma_start(gnb_sb[:], gnb_ap.unsqueeze(0))

            # resident attention operands
            kT = big.tile([128, CT, N], att_dt, tag="kT")
            qT = res.tile([128, CT, N], att_dt, tag="qT")
            vtm = res.tile([128, NT, C], att_dt, tag="v")

            w8 = {}
            if qkv8:
                for name in ("wq", "wk", "wv"):
                    w8[name] = consts.tile([128, CT, C], FP8, tag=f"{name}8",
                                           name=f"{name}8")

            with tc.tile_pool(name="hTp", bufs=1) as hTp:
                hT = hTp.tile([128, CT, N], h_dt, tag="hT")

                # ---- phase A: load x, stats, transpose -------------------
                xr_all = big.tile([128, NT, C], BF16, tag="xr")
                with (
                    tc.tile_pool(name="pa_ps", bufs=1, space=bass.MemorySpace.PSUM) as paps,
                    tc.tile_pool(name="pa_tps", bufs=4, space=bass.MemorySpace.PSUM) as patps,
                    tc.tile_pool(name="xin", bufs=3) as xin,
                    tc.tile_pool(name="x2p", bufs=1) as x2p,
                ):
                    s1_ps = paps.tile([1, C], F32, tag="s1")
                    s2_ps = paps.tile([1, C], F32, tag="s2")
                    CH = 4  # nt per chunk
                    for ch in range(NT // CH):
                        x_t = xin.tile([128, CH, C], F32, tag="x_t")
                        nc.sync.dma_start(
                            x_t[:], x_r[ch * CH:(ch + 1) * CH].transpose([1, 0, 2]))
                        xr_t = xr_all[:, ch * CH:(ch + 1) * CH, :]
                        nc.vector.tensor_copy(xr_t, x_t[:])
                        x2_t = x2p.tile([128, CH, C], BF16, tag="x2_t")
                        nc.scalar.activation(x2_t[:], x_t[:], AF.Square)
                        for u in range(CH):
                            nt = ch * CH + u
                            nc.tensor.matmul(s1_ps[:], ones_bf[:], xr_all[:, nt, :],
                                             start=(nt == 0), stop=(nt == NT - 1))
                            nc.tensor.matmul(s2_ps[:], ones_bf[:], x2_t[:, u, :],
                                             start=(nt == 0), stop=(nt == NT - 1))
                        # transpose this chunk (4 nt x 4 ct) while the next
                        # chunk's DMA is in flight
                        for ct in range(CT):
                            tp = patps.tile([128, 512], BF16, tag="tp")
                            for u in range(CH):
                                nt = ch * CH + u
                                nc.tensor.transpose(
                                    tp[:, bass.ts(u, 128)],
                                    xr_all[:, nt, bass.ts(ct, 128)], id_bf[:])
                            nc.vector.tensor_copy(
                                hT[:, ct, ch * 512:(ch + 1) * 512], tp[:])

                    # group stats on partition 0
                    g1 = small.tile([1, G], F32, tag="g1")
                    nc.vector.reduce_sum(
                        g1[:], s1_ps[:].rearrange("p (g k) -> p g k", k=CPG),
                        axis=mybir.AxisListType.X)
                    g2 = small.tile([1, G], F32, tag="g2")
                    nc.vector.reduce_sum(
                        g2[:], s2_ps[:].rearrange("p (g k) -> p g k", k=CPG),
                        axis=mybir.AxisListType.X)
                    cnt = 1.0 / (N * CPG)
                    mean = small.tile([1, G], F32, tag="mean")
                    nc.scalar.mul(mean[:], g1[:], cnt)
                    ex2 = small.tile([1, G], F32, tag="ex2")
                    nc.scalar.mul(ex2[:], g2[:], cnt)
                    var = small.tile([1, G], F32, tag="var")
                    nc.vector.tensor_tensor(var[:], mean[:], mean[:], op=ALU.mult)
                    nc.vector.tensor_tensor(var[:], ex2[:], var[:], op=ALU.subtract)
                    eps_t = small.tile([1, 1], F32, tag="eps_t")
                    nc.vector.memset(eps_t[:], EPS)
                    sd = small.tile([1, G], F32, tag="sd")
                    nc.scalar.activation(sd[:], var[:], AF.Sqrt, bias=eps_t[:])
                    inv = small.tile([1, G], F32, tag="inv")
                    nc.vector.reciprocal(inv[:], sd[:])
                    # broadcast group -> channel (free-dim stride-0 read)
                    invc = small.tile([1, C], F32, tag="invc")
                    nc.vector.tensor_copy(
                        invc[:].rearrange("p (g k) -> p g k", k=CPG),
                        inv[:].unsqueeze(2).broadcast_to([1, G, CPG]))
                    meanc = small.tile([1, C], F32, tag="meanc")
                    nc.vector.tensor_copy(
                        meanc[:].rearrange("p (g k) -> p g k", k=CPG),
                        mean[:].unsqueeze(2).broadcast_to([1, G, CPG]))
                    a_c = small.tile([1, C], F32, tag="a_c")
                    nc.vector.tensor_tensor(a_c[:], invc[:], gns_sb[:], op=ALU.mult)
                    b_c = small.tile([1, C], F32, tag="b_c")
                    nc.vector.tensor_tensor(b_c[:], meanc[:], a_c[:], op=ALU.mult)
                    nc.vector.tensor_tensor(b_c[:], gnb_sb[:], b_c[:], op=ALU.subtract)
                    # bounce [1, C] -> per-partition [128, 2, CT]
                    nc.sync.dma_start(gn_bounce[0].unsqueeze(0), a_c[:])
                    nc.sync.dma_start(gn_bounce[1].unsqueeze(0), b_c[:])
                    ab_sb = small.tile([128, 2, CT], F32, tag="ab_sb")
                    nc.sync.dma_start(
                        ab_sb[:], gn_bounce.rearrange("two (ct p) -> p two ct", p=128))
                    b_rT = small.tile([128, CT], BF16, tag="b_rT")
                    nc.vector.tensor_copy(b_rT[:], ab_sb[:, 1, :])
                    # scaled weights in-place: w = a_c (row scale) * w.
                    # Tile orders these after the bias matmuls below, which
                    # read the raw weights (WAR on the same tile).
                    def _scale_weights():
                        for name in ("wq", "wk", "wv"):
                            for ct in range(CT):
                                dst = (w8[name][:, ct, :] if qkv8
                                       else w_bf[name][:, ct, :])
                                nc.scalar.activation(
                                    dst, w_bf[name][:, ct, :],
                                    AF.Copy, bias=0.0,
                                    scale=ab_sb[:, 0, ct:ct + 1])
                    # effective biases: b_c @ w + orig_bias
                    with tc.tile_pool(name="bps", bufs=1,
                                      space=bass.MemorySpace.PSUM) as bps:
                        for i, name in enumerate(("wq", "wk", "wv")):
                            bp = bps.tile([1, C], F32, tag="bp", name=f"bp{i}")
                            for ct in range(CT):
                                nc.tensor.matmul(
                                    bp[:], b_rT[:, ct:ct + 1], w_bf[name][:, ct, :],
                                    start=(ct == 0), stop=(ct == CT - 1))
                            btmp = small.tile([1, C], F32, tag="btmp",
                                              name=f"btmp{i}")
                            nc.vector.tensor_copy(btmp[:], bp[:])
                            nc.sync.dma_start(bias_bounce[i].unsqueeze(0), btmp[:])
                            if name == "wv":
                                nc.sync.dma_start(
                                    bvb[:],
                                    bias_bounce[i].unsqueeze(0).partition_broadcast(128))
                                nc.vector.tensor_tensor(bvb[:], bvb[:],
                                                        bv_direct[:], op=ALU.add)
                            else:
                                nc.sync.dma_start(
                                    bqkT[:, i, :],
                                    bias_bounce[i].rearrange("(dt p) -> p dt", p=128))
                                nc.vector.tensor_tensor(
                                    bqkT[:, i, :], bqkT[:, i, :],
                                    bqk_direct[:, i, :], op=ALU.add)
                        _scale_weights()

                # ---- phase B: q^T, k^T, v (all SBUF-resident) -------------
                if "B" in phases:
                  with (
                    tc.tile_pool(name="pb_ps", bufs=4, space=bass.MemorySpace.PSUM) as pbps,
                  ):
                    def _qk_mms(ps, wname, nb2, dt, half):
                        lo = nb2 * 1024 + half * 512
                        if qkv8:
                            for cp in range(CT // 2):
                                nc.tensor.matmul(
                                    ps[:, bass.ts(half, 512)],
                                    w8[wname][:, 2 * cp:2 * cp + 2, bass.ts(dt, 128)],
                                    hT[:, 2 * cp:2 * cp + 2, lo:lo + 512],
                                    start=(cp == 0), stop=(cp == CT // 2 - 1),
                                    perf_mode=DR)
                        else:
                            for ct in range(CT):
                                nc.tensor.matmul(
                                    ps[:, bass.ts(half, 512)],
                                    w_bf[wname][:, ct, bass.ts(dt, 128)],
                                    hT[:, ct, lo:lo + 512],
                                    start=(ct == 0), stop=(ct == CT - 1))

                    for nb2 in range(N // 1024):
                        for dt in range(CT):
                            k_ps = pbps.tile([128, 1024], F32, tag="qkv_ps")
                            for half in range(2):
                                _qk_mms(k_ps, "wk", nb2, dt, half)
                            nc.vector.tensor_scalar(
                                kT[:, dt, bass.ts(nb2, 1024)], k_ps[:],
                                1.0, bqkT[:, 1, dt:dt + 1],
                                op0=ALU.mult, op1=ALU.add)

                            q_ps = pbps.tile([128, 1024], F32, tag="qkv_ps")
                            for half in range(2):
                                _qk_mms(q_ps, "wq", nb2, dt, half)
                            nc.scalar.activation(qT[:, dt, bass.ts(nb2, 1024)],
                                                 q_ps[:], AF.Identity,
                                                 bias=bqkT[:, 0, dt:dt + 1])
                    for nt2 in range(NT // 2):
                        v_ps = pbps.tile([128, 1024], F32, tag="qkv_ps")
                        for half in range(2):
                            nt = nt2 * 2 + half
                            if qkv8:
                                for cp in range(CT // 2):
                                    nc.tensor.matmul(
                                        v_ps[:, bass.ts(half, 512)],
                                        hT[:, 2 * cp:2 * cp + 2, bass.ts(nt, 128)],
                                        w8["wv"][:, 2 * cp:2 * cp + 2, :],
                                        start=(cp == 0), stop=(cp == CT // 2 - 1),
                                        perf_mode=DR)
                            else:
                                for ct in range(CT):
                                    nc.tensor.matmul(
                                        v_ps[:, bass.ts(half, 512)],
                                        hT[:, ct, bass.ts(nt, 128)],
                                        w_bf["wv"][:, ct, :],
                                        start=(ct == 0), stop=(ct == CT - 1))
                        nc.vector.tensor_tensor(
                            vtm[:, nt2 * 2:nt2 * 2 + 2, :],
                            v_ps[:].rearrange("p (two c) -> p two c", two=2),
                            bvb[:].unsqueeze(1).broadcast_to([128, 2, C]),
                            op=ALU.add)

            # ---- phase C: attention + proj + residual ----------------------
            if "C" not in phases:
                with tc.tile_pool(name="dummy_out", bufs=1) as dop:
                    d_t = dop.tile([128, C], F32, tag="d_t")
                    nc.vector.memset(d_t[:], 0.0)
                    for nt in range(NT):
                        nc.sync.dma_start(out_r[nt], d_t[:])
            from collections import deque
            if "C" in phases:
              with (
                tc.tile_pool(name="pc_s", bufs=3, space=bass.MemorySpace.PSUM) as pcs,
                tc.tile_pool(name="pc_av", bufs=CT, space=bass.MemorySpace.PSUM) as pcav,
                tc.tile_pool(name="pc_o", bufs=1, space=bass.MemorySpace.PSUM) as pco,
                tc.tile_pool(name="atp", bufs=4) as atp,
                tc.tile_pool(name="avtp", bufs=2) as avtp,
                tc.tile_pool(name="xbp", bufs=4) as xbp,
                tc.tile_pool(name="obp", bufs=3) as obp,
                tc.tile_pool(name="rp", bufs=2) as rp,
            ):
                def _make_finalize(g, avT, r_sb):
                    # proj + residual for group g, issued early in group g+1
                    # so the PE never stalls on the avT copies
                    def _fin():
                        for nb in range(NB):
                            o_ps = pcs.tile([128, C], F32, tag="s_ps",
                                            name=f"o_ps_{g}_{nb}")
                            for dt in range(CT):
                                nc.tensor.matmul(
                                    o_ps[:],
                                    avT[:, dt, bass.ts(nb, 128)],
                                    w_bf["wo"][:, dt, :],
                                    start=(dt == 0), stop=(dt == CT - 1))
                            nt = g * NB + nb
                            o_sb = obp.tile([128, C], F32, tag="o_sb")
                            nc.vector.tensor_copy(o_sb[:], o_ps[:])
                            xb = xbp.tile([128, C], F32, tag="xb")
                            nc.sync.dma_start(xb[:], x_r[nt])
                            nc.vector.tensor_tensor(xb[:], xb[:], bob[:],
                                                    op=ALU.add)
                            ob = obp.tile([128, C], F32, tag="ob")
                            nc.vector.scalar_tensor_tensor(
                                ob[:], o_sb[:], r_sb[:, nb:nb + 1], xb[:],
                                op0=ALU.mult, op1=ALU.add)
                            nc.sync.dma_start(out_r[nt], ob[:])
                    return _fin

                prev_fin = None
                for g in range(NG):
                    av_ps = [pcav.tile([128, GW], F32, tag="av", name=f"av_ps{dt}")
                             for dt in range(CT)]
                    den_ps = pco.tile([1, GW], F32, tag="den_ps", name="den_ps")
                    pend_q = deque()

                    if attn.startswith("fp8"):
                        def _issue_av(pa, pj, last):
                            nc.tensor.matmul(den_ps[:], ones8, pa[:],
                                             start=(pj == 0), stop=last,
                                             perf_mode=DR)
                            for dt in range(CT):
                                nc.tensor.matmul(
                                    av_ps[dt][:],
                                    vtm[:, 2 * pj:2 * pj + 2, bass.ts(dt, 128)],
                                    pa[:],
                                    start=(pj == 0), stop=last,
                                    perf_mode=DR)

                        for jtp in range(NT // 2):
                            a_pair = atp.tile([128, 2, GW], FP8, tag="a_pair")
                            for u2 in range(2):
                                jt = 2 * jtp + u2
                                s_ps = pcs.tile([128, GW], F32, tag="s_ps")
                                for cp in range(CT // 2):
                                    nc.tensor.matmul(
                                        s_ps[:],
                                        kT[:, 2 * cp:2 * cp + 2, bass.ts(jt, 128)],
                                        qT[:, 2 * cp:2 * cp + 2, bass.ts(g, GW)],
                                        start=(cp == 0), stop=(cp == 1),
                                        perf_mode=DR)
                                nc.scalar.activation(a_pair[:, u2, :], s_ps[:],
                                                     AF.Exp,
                                                     scale=float(C) ** -0.5,
                                                     bias=eb_t[:])
                            pend_q.append((a_pair, jtp))
                            if jtp == 2 and prev_fin is not None:
                                prev_fin()
                                prev_fin = None
                            if len(pend_q) > 2:
                                pa, pj = pend_q.popleft()
                                _issue_av(pa, pj, False)
                        while pend_q:
                            pa, pj = pend_q.popleft()
                            _issue_av(pa, pj, not pend_q)
                    else:
                        def _issue_av16(pa, pj, last):
                            nc.tensor.matmul(den_ps[:], ones_bf[:], pa[:],
                                             start=(pj == 0), stop=last)
                            for dt in range(CT):
                                nc.tensor.matmul(
                                    av_ps[dt][:],
                                    vtm[:, pj, bass.ts(dt, 128)],
                                    pa[:],
                                    start=(pj == 0), stop=last)

                        for jt in range(NT):
                            s_ps = pcs.tile([128, GW], F32, tag="s_ps")
                            for ct in range(CT):
                                nc.tensor.matmul(
                                    s_ps[:],
                                    kT[:, ct, bass.ts(jt, 128)],
                                    qT[:, ct, bass.ts(g, GW)],
                                    start=(ct == 0), stop=(ct == CT - 1))
                            a_t = atp.tile([128, GW], BF16, tag="a_pair")
                            nc.scalar.activation(a_t[:], s_ps[:], AF.Exp,
                                                 scale=float(C) ** -0.5)
                            pend_q.append((a_t, jt))
                            if jt == 4 and prev_fin is not None:
                                prev_fin()
                                prev_fin = None
                            if len(pend_q) > 3:
                                pa, pj = pend_q.popleft()
                                _issue_av16(pa, pj, False)
                        while pend_q:
                            pa, pj = pend_q.popleft()
                            _issue_av16(pa, pj, not pend_q)

                    # reciprocal -> per-partition via DRAM bounce
                    recip = rp.tile([1, GW], F32, tag="recip")
                    nc.vector.reciprocal(recip[:], den_ps[:])
                    nc.sync.dma_start(den_bounce[g].unsqueeze(0), recip[:])
                    r_sb = rp.tile([128, NB], F32, tag="r_sb")
                    nc.sync.dma_start(
                        r_sb[:], den_bounce[g].rearrange("(nb p) -> p nb", p=128))
                    # AV^T -> SBUF (bf16) for proj lhsT
                    avT = avtp.tile([128, CT, GW], BF16, tag="avT")
                    for dt in range(CT):
                        nc.vector.tensor_copy(avT[:, dt, :], av_ps[dt][:])
                    prev_fin = _make_finalize(g, avT, r_sb)
                prev_fin()

    nc.compile()
    return nc


_CACHE = {}


def _get_program(reps: int = 1, attn: str = "fp8", phases: str = "ABC"):
    key = (reps, attn, phases)
    if key not in _CACHE:
        _CACHE[key] = build_program(reps, attn, phases)
    return _CACHE[key]


def make_in_maps(inputs):
    ident = np.eye(128, dtype=np.float32)
    x = np.asarray(inputs["x"], dtype=np.float32).reshape(B, N, C)
    shared = {k: np.ascontiguousarray(np.asarray(inputs[k], dtype=np.float32))
              for k in ("wq", "wk", "wv", "wo", "bq", "bk", "bv", "bo",
                        "gn_scale", "gn_bias")}
    return [dict(x=np.ascontiguousarray(x[c]), ident=ident, **shared)
            for c in range(N_CORES)]


DEFAULT_ATTN = "fp8x"


def kernel(**inputs) -> np.ndarray:
    nc = _get_program(1, DEFAULT_ATTN)
    in_maps = make_in_maps(inputs)
    last_err = None
    for _attempt in range(3):
        try:
            res = run_bass_kernel_spmd(nc, in_maps, list(range(N_CORES)))
            break
        except Exception as e:  # transient NRT device errors recover on retry
            last_err = e
    else:
        raise last_err
    out = np.stack([res.results[c]["out"] for c in range(N_CORES)], axis=0)
    return out.reshape(B, H, W, C)
